# revision 11
# baseline (speedup 1.0000x reference)
"""Trainium2 Bass kernel for nn_AssociationScore (GCN + MLP scoring head).

The computation is linear up to the final sigmoid, so the 64-dim GCN
aggregation collapses to a per-node scalar:
    w3  = W @ w2                       (256-vector, computed on device)
    u   = x @ w3                       (per-node scalar; memory-bound matvec)
    g   = u * dinv                     (dinv = rsqrt(indeg + 1))
    z0[d] = sum over edges (s->d) of g[s]
    score = sigmoid(dinv*z0 + dinv^2*u + (b@w2 + b2))

Sharding: nodes row-sharded over 8 NeuronCores (12500/core). Each core
computes its u/g shard, all-gathers g, then aggregates the edges whose dst
lies in its shard. The scatter-sum runs as: per-source-octant GPSIMD
ap_gather of g (8 independent index streams, one per Q7 core) -> DVE prefix
scan along the free dim -> second ap_gather at segment-boundary positions ->
differences. Host-side work is integer routing only (sort + CSR offsets +
int16 index tables); all floating-point math runs on device.
"""
import numpy as np

NCORES = 8
N = 100000
M = 12500            # nodes per core
TILES = 98           # 98 * 128 = 12544
MP = TILES * 128
D = 256
H = 64
NCH = 8              # gather/scan pipeline chunks
TB = 7               # matvec tiles per block
_GATE = [0] * 7      # set by _routing: scan chunk gating each extraction (98 = 14*7)


def _routing(src, dst):
    """Integer routing tables. Returns per-core wrapped int16 index arrays and sizes."""
    core = dst // M
    octv = src // M
    key = (core * 8 + octv) * np.int64(N) + dst
    order = np.argsort(key, kind='stable')
    s_s, d_s = src[order], dst[order]
    grp = (core * 8 + octv)[order]
    counts = np.bincount(grp, minlength=64)
    KE = int(counts.max()) + 1                      # +1 sentinel at stream head
    KE = ((KE + NCH * 64 - 1) // (NCH * 64)) * (NCH * 64)
    assert KE <= 32768
    offs = np.concatenate([[0], np.cumsum(counts)])
    idx_main = np.full((NCORES, 8, KE), M, np.int16)    # sentinel idx = M (zero pad row)
    ends = np.zeros((NCORES, 8, MP), np.int64)
    for c in range(NCORES):
        for o in range(8):
            gi = c * 8 + o
            L = counts[gi]
            sl = slice(offs[gi], offs[gi] + L)
            idx_main[c, o, 1:1 + L] = (s_s[sl] - o * M).astype(np.int16)
            dl = d_s[sl] - c * M
            cnt = np.bincount(dl, minlength=M)
            e = np.cumsum(cnt)                      # stream pos of last edge with dst<=n
            ends[c, o, :M] = e
            ends[c, o, M:] = e[-1]
    EKE = ((MP + TILES + 63) // 64) * 64
    ext = np.zeros((NCORES, 8, EKE), np.int64)
    ext[:, :, :MP] = ends
    for j in range(1, TILES):
        ext[:, :, MP + j] = ends[:, :, 128 * j - 1]
    deg = (np.bincount(dst, minlength=N) + 1).astype(np.float32)
    # extraction gating: node-chunk j (1792 nodes, 14 tiles) may extract only
    # after the scan chunk covering its largest end position completes
    CH = KE // NCH
    global _GATE
    _GATE = [max(0, int(ends[:, :, 1792 * (j + 1) - 1].max() - 1) // CH)
             for j in range(7)]

    def wrap16(a):                                  # [8, K] -> [128, K//16]
        o, K = a.shape
        return np.ascontiguousarray(
            a.reshape(o, K // 16, 16).transpose(0, 2, 1).reshape(o * 16, K // 16))

    idxm = np.stack([wrap16(idx_main[c]) for c in range(NCORES)])
    idxe = np.stack([wrap16(ext[c].astype(np.int16)) for c in range(NCORES)])
    return idxm, idxe, deg, KE, EKE


def _emit(nc, tc, t, KE, EKE, variant=9):
    """Emit the per-core program. `t` maps tensor names to DRAM APs."""
    import concourse.mybir as mybir
    import concourse.tile as tile  # noqa: F401

    dt = mybir.dt
    f32 = dt.float32
    Alu = mybir.AluOpType

    with tc.tile_pool(name="const", bufs=1) as cpool, \
         tc.tile_pool(name="big", bufs=1) as bpool, \
         tc.tile_pool(name="scr", bufs=1) as spool, \
         tc.tile_pool(name="gchp", bufs=2) as gpool, \
         tc.tile_pool(name="ps", bufs=1, space="PSUM") as ppool, \
         tc.tile_pool(name="psp", bufs=2, space="PSUM") as prodpool, \
         tc.tile_pool(name="dram", bufs=1, space="DRAM") as dpool:

        # ---- constants
        identt = cpool.tile([128, 128], f32)
        nc.sync.dma_start(identt[:], t["ident"])
        mselt = cpool.tile([128, 128], f32)
        nc.sync.dma_start(mselt[:], t["msel"])
        wTt = cpool.tile([H, D], f32)
        nc.sync.dma_start(wTt[:], t["wT"])
        w2rt = cpool.tile([H, 128], f32)
        nc.sync.dma_start(w2rt[:], t["w2r"])
        bcolt = cpool.tile([H, 1], f32)
        nc.sync.dma_start(bcolt[:], t["bcol"])
        b2rt = cpool.tile([128, 1], f32)
        nc.sync.dma_start(b2rt[:], t["b2r"])
        idxmt = cpool.tile([128, KE // 16], dt.int16)
        nc.sync.dma_start(idxmt[:], t["idxm"])
        idxet = cpool.tile([128, EKE // 16], dt.int16)
        nc.sync.dma_start(idxet[:], t["idxe"])
        degt = cpool.tile([TILES, 128], f32)
        nc.sync.dma_start(degt[:], t["deg2d"])

        if variant <= 0:
            nc.sync.dma_start(t["out"].rearrange("(a b) -> a b", a=TILES), degt[:])
            return

        # ---- w3 replicated [128, D]: out[p,k] = sum_j w2r[j,p]*wT[j,k]
        w3ps = ppool.tile([128, D], f32, tag="w3ps")
        nc.tensor.matmul(w3ps[:], w2rt[:], wTt[:], start=True, stop=True)
        w3rep = cpool.tile([128, D], f32)
        nc.vector.tensor_copy(w3rep[:], w3ps[:])
        if variant == 1:   # w3rep matmul only
            nc.sync.dma_start(t["out"].rearrange("(a b) -> a b", a=TILES), w3rep[0:TILES, 0:128])
            return

        # ---- c0 = b@w2 + b2, replicated [128, 1]
        c0ps = ppool.tile([128, 1], f32, tag="c0ps")
        nc.tensor.matmul(c0ps[:], w2rt[:], bcolt[:], start=True, stop=True)
        if variant == 11:   # + c0 matmul, no add
            nc.sync.dma_start(t["out"].rearrange("(a b) -> a b", a=TILES), w3rep[0:TILES, 0:128])
            return
        c0t = cpool.tile([128, 1], f32)
        nc.vector.tensor_add(c0t[:], c0ps[:], b2rt[:])
        if variant == 12:   # + c0 add
            nc.sync.dma_start(t["out"].rearrange("(a b) -> a b", a=TILES), w3rep[0:TILES, 0:128])
            return

        if variant == 2:   # consts + matmul w3rep only
            res0 = spool.tile([128, D], f32, tag="prodc")
            nc.vector.tensor_copy(res0[:], w3rep[:])
            nc.sync.dma_start(t["out"].rearrange("(a b) -> a b", a=TILES), res0[0:TILES, 0:128])
            return

        # ---- matvec u = xs @ w3 -> U [128, TILES]  (TB tiles per block)
        U = bpool.tile([128, 128], f32)
        nc.vector.memset(U[:], 0.0)
        NB = 1 if variant == 3 else TILES // TB
        xsv = t["xs"].rearrange("(b a p) k -> b p a k", p=128, a=TB)
        with tc.tile_pool(name="xload", bufs=2) as xpool:
            for j in range(NB):
                xt = xpool.tile([128, TB * D], f32, tag="xt")
                nc.sync.dma_start(
                    xt[:].rearrange("p (a k) -> p a k", k=D), xsv[j])
                xt3 = xt[:].rearrange("p (a k) -> p a k", k=D)
                nc.vector.tensor_mul(
                    xt3, xt3,
                    w3rep[:].rearrange("p (o k) -> p o k", o=1)
                    .broadcast_to([128, TB, D]))
                nc.vector.tensor_reduce(
                    U[:, j * TB:(j + 1) * TB],
                    xt[:].rearrange("p (a k) -> p a k", k=D),
                    axis=mybir.AxisListType.X, op=Alu.add)

        if variant in (3, 4):   # matvec only (1 tile or all)
            nc.sync.dma_start(t["out"].rearrange("(a b) -> a b", a=TILES), U[0:TILES, :])
            return

        # ---- dinv = sqrt(1/deg) [TILES, 128]
        rec = spool.tile([TILES, 128], f32, tag="rec")
        nc.vector.reciprocal(rec[:], degt[:])
        dinvt = bpool.tile([TILES, 128], f32)
        nc.scalar.sqrt(dinvt[:], rec[:])

        # ---- transpose U -> [TILES, 128]; g = dinv * uT
        ups = ppool.tile([128, 128], f32, tag="ups")
        nc.tensor.transpose(ups[:], U[:], identt[:])
        gblk = bpool.tile([TILES, 128], f32)
        nc.vector.tensor_mul(gblk[:], dinvt[:], ups[0:TILES, :])

        if variant <= 5:
            res1 = spool.tile([TILES, 128], f32, tag="res")
            nc.scalar.activation(res1[:], gblk[:],
                                 mybir.ActivationFunctionType.Sigmoid,
                                 bias=c0t[0:TILES, :])
            nc.sync.dma_start(t["out"].rearrange("(a b) -> a b", a=TILES), res1[:])
            return

        # ---- allgather g
        gshard = dpool.tile([1, MP], f32)
        nc.sync.dma_start(
            gshard.opt()[0].rearrange("(a b) -> a b", a=TILES), gblk[:])
        gfull = dpool.tile([NCORES, MP], f32)
        nc.gpsimd.collective_compute(
            "AllGather", Alu.bypass,
            replica_groups=[list(range(NCORES))],
            ins=[gshard.opt()], outs=[gfull.opt()])

        # ---- octant tables [128, MP]: partition group o holds shard o (x16)
        gtab = bpool.tile([128, MP], f32, tag="gtb")
        for o in range(8):
            try:
                src = gfull.opt()[o:o + 1, :].broadcast_to([16, MP])
                nc.sync.dma_start(gtab[16 * o:16 * (o + 1), :], src)
            except Exception:
                for r in range(16):
                    nc.sync.dma_start(
                        gtab[16 * o + r:16 * o + r + 1, :],
                        gfull.opt()[o:o + 1, :])

        if variant <= 6:
            res2 = spool.tile([TILES, 128], f32, tag="res")
            nc.vector.tensor_copy(res2[:], gtab[0:TILES, 0:128])
            nc.sync.dma_start(t["out"].rearrange("(a b) -> a b", a=TILES), res2[:])
            return

        # ---- gather + prefix scan over edge streams; the boundary
        # extraction runs in 7 node-chunks (1792 nodes = 14 tiles each),
        # gated on scan progress and interleaved with the gathers. Octant
        # partials are summed on the PE via a row-selector matmul, so no
        # DRAM bounce is needed.
        scn = bpool.tile([128, KE], f32)
        ones1 = cpool.tile([128, 1], f32)
        nc.vector.memset(ones1[:], 1.0)
        CH = KE // NCH
        EXN = 1792
        scn3 = scn[:].rearrange("p (n d) -> p n d", d=1)
        gtab3 = gtab[:].rearrange("p (n d) -> p n d", d=1)
        Zt = bpool.tile([TILES, 128], f32)
        zred = dpool.tile([1, 7 * 1792], f32)

        def emit_ext(j):
            BOj = gpool.tile([128, CH], f32, tag="gch")
            nc.gpsimd.ap_gather(
                out_ap=BOj[:, 0:EXN].rearrange("p (n d) -> p n d", d=1),
                in_ap=scn3,
                idxs_ap=idxet[:, (EXN // 16) * j:(EXN // 16) * (j + 1)],
                channels=128, num_elems=KE, d=1,
                num_idxs=EXN)
            # sum the 8 octant rows (partitions 16o) of each 128-col block:
            # out[p, f] = sum_c msel[c, p] * BOj[c, f] = sum_o BOj[16o, f]
            ZSj = gpool.tile([128, CH], f32, tag="gch")
            for m2 in range(EXN // 128):
                zm = prodpool.tile([128, 128], f32, tag="zm")
                nc.tensor.matmul(zm[:], mselt[:],
                                 BOj[:, 128 * m2:128 * (m2 + 1)],
                                 start=True, stop=True)
                nc.vector.tensor_copy(ZSj[:, 128 * m2:128 * (m2 + 1)], zm[:])
            # redistribute [1, 1792] (rows identical) -> Zt rows [14j, 14j+14)
            # via a DRAM bounce (SBUF partition-folding reads diverge on HW)
            nc.sync.dma_start(zred.opt()[0:1, EXN * j:EXN * (j + 1)],
                              ZSj[0:1, 0:EXN])
            nc.sync.dma_start(
                Zt[14 * j:14 * (j + 1), :],
                zred.opt()[0, EXN * j:EXN * (j + 1)]
                .rearrange("(a b) -> a b", b=128))

        nxj = 0
        for tt in range(NCH):
            gch = gpool.tile([128, CH], f32, tag="gch")
            nc.gpsimd.ap_gather(
                out_ap=gch[:].rearrange("p (n d) -> p n d", d=1),
                in_ap=gtab3,
                idxs_ap=idxmt[:, tt * (CH // 16):(tt + 1) * (CH // 16)],
                channels=128, num_elems=MP, d=1,
                num_idxs=CH)
            if variant <= 7:
                res3 = spool.tile([TILES, 128], f32, tag="res")
                nc.vector.tensor_copy(res3[:], gch[0:TILES, 0:128])
                nc.sync.dma_start(t["out"].rearrange("(a b) -> a b", a=TILES), res3[:])
                return
            init = 0.0 if tt == 0 else scn[:, tt * CH - 1:tt * CH]
            nc.vector.tensor_tensor_scan(
                out=scn[:, tt * CH:(tt + 1) * CH],
                data0=ones1[:].broadcast_to([128, CH]),
                data1=gch[:],
                initial=init, op0=Alu.mult, op1=Alu.add)
            if variant > 8:
                while nxj < 7 and _GATE[nxj] <= tt - 1:
                    emit_ext(nxj)
                    nxj += 1

        if variant <= 8:
            res4 = spool.tile([TILES, 128], f32, tag="res")
            nc.vector.tensor_copy(res4[:], scn[0:TILES, 0:128])
            nc.sync.dma_start(t["out"].rearrange("(a b) -> a b", a=TILES), res4[:])
            return

        while nxj < 7:
            emit_ext(nxj)
            nxj += 1

        # ---- diff, combine, sigmoid. Tile-row boundary values come from a
        # one-column shift DMA of Zt itself (Zx[t] = Zt[t-1, 127], Zx[0] = 0).
        Zx = spool.tile([TILES, 1], f32, tag="zx")
        nc.vector.memset(Zx[:], 0.0)
        nc.sync.dma_start(Zx[1:TILES, 0:1], Zt[0:TILES - 1, 127:128])
        Dt = spool.tile([TILES, 128], f32, tag="dt")
        nc.vector.tensor_sub(Dt[:, 1:128], Zt[:, 1:128], Zt[:, 0:127])
        nc.vector.tensor_sub(Dt[:, 0:1], Zt[:, 0:1], Zx[:])
        t1 = spool.tile([TILES, 128], f32, tag="t1")
        nc.vector.tensor_add(t1[:], Dt[:], gblk[:])
        t2 = spool.tile([TILES, 128], f32, tag="t2")
        nc.vector.tensor_mul(t2[:], dinvt[:], t1[:])
        res = spool.tile([TILES, 128], f32, tag="res")
        nc.scalar.activation(res[:], t2[:],
                             mybir.ActivationFunctionType.Sigmoid,
                             bias=c0t[0:TILES, :])
        nc.sync.dma_start(t["out"].rearrange("(a b) -> a b", a=TILES), res[:])


def _build_nc(KE, EKE, variant=9):
    import concourse.bacc as bacc
    import concourse.mybir as mybir
    import concourse.tile as tile

    dt = mybir.dt
    f32 = dt.float32
    nc = bacc.Bacc("TRN2", target_bir_lowering=False, debug=False,
                   num_devices=NCORES)
    t = {
        "xs": nc.dram_tensor("xs", [MP, D], f32, kind="ExternalInput").ap(),
        "wT": nc.dram_tensor("wT", [H, D], f32, kind="ExternalInput").ap(),
        "w2r": nc.dram_tensor("w2r", [H, 128], f32, kind="ExternalInput").ap(),
        "bcol": nc.dram_tensor("bcol", [H, 1], f32, kind="ExternalInput").ap(),
        "b2r": nc.dram_tensor("b2r", [128, 1], f32, kind="ExternalInput").ap(),
        "deg2d": nc.dram_tensor("deg2d", [TILES, 128], f32, kind="ExternalInput").ap(),
        "ident": nc.dram_tensor("ident", [128, 128], f32, kind="ExternalInput").ap(),
        "msel": nc.dram_tensor("msel", [128, 128], f32, kind="ExternalInput").ap(),
        "idxm": nc.dram_tensor("idxm", [128, KE // 16], dt.int16, kind="ExternalInput").ap(),
        "idxe": nc.dram_tensor("idxe", [128, EKE // 16], dt.int16, kind="ExternalInput").ap(),
        "out": nc.dram_tensor("out", [MP], f32, kind="ExternalOutput").ap(),
    }
    with tile.TileContext(nc) as tc:
        _emit(nc, tc, t, KE, EKE, variant)
    nc.compile()
    return nc


def _make_in_maps(x, edge_index, W, b, w2, b2):
    src = np.asarray(edge_index[0], dtype=np.int64)
    dst = np.asarray(edge_index[1], dtype=np.int64)
    idxm, idxe, deg, KE, EKE = _routing(src, dst)

    xf = np.asarray(x, dtype=np.float32)
    Wf = np.asarray(W, dtype=np.float32)
    w2f = np.asarray(w2, dtype=np.float32).reshape(H)
    bf = np.asarray(b, dtype=np.float32)
    b2f = np.asarray(b2, dtype=np.float32).reshape(1)

    wT = np.ascontiguousarray(Wf.T)                       # [H, D]
    w2rep = np.ascontiguousarray(np.broadcast_to(w2f.reshape(H, 1), (H, 128)))
    bcol = bf.reshape(H, 1)
    b2rep = np.full((128, 1), float(b2f[0]), np.float32)
    identm = np.eye(128, dtype=np.float32)
    mselm = np.zeros((128, 128), np.float32)
    mselm[0::16, :] = 1.0          # matmul(out, msel, B): out[p,f]=sum_o B[16o,f]

    in_maps = []
    for c in range(NCORES):
        xsp = np.zeros((MP, D), np.float32)
        xsp[:M] = xf[c * M:(c + 1) * M]
        degp = np.ones(MP, np.float32)
        degp[:M] = deg[c * M:(c + 1) * M]
        in_maps.append({
            "xs": xsp,
            "wT": wT,
            "w2r": w2rep,
            "bcol": bcol,
            "b2r": b2rep,
            "deg2d": degp.reshape(TILES, 128),
            "ident": identm,
            "msel": mselm,
            "idxm": idxm[c],
            "idxe": idxe[c],
        })
    return in_maps, KE, EKE


def _unshard(outv):
    return np.concatenate([outv[c][:M] for c in range(NCORES)]).astype(np.float32)


def kernel(x, edge_index, W, b, w2, b2):
    in_maps, KE, EKE = _make_in_maps(x, edge_index, W, b, w2, b2)
    nc = _build_nc(KE, EKE)
    from concourse.bass_utils import run_bass_kernel_spmd
    res = None
    for attempt in range(3):
        try:
            res = run_bass_kernel_spmd(nc, in_maps, list(range(NCORES)))
            break
        except Exception:
            if attempt == 2:
                raise
            # device may be in a bad state from a prior run; exercise the
            # plain XLA path on every core to recover, then retry
            import jax
            import jax.numpy as jnp
            a = np.eye(128, dtype=np.float32)
            for d in jax.devices()[:NCORES]:
                jnp.dot(jax.device_put(a, d), jax.device_put(a, d)).block_until_ready()
    outv = np.stack([res.results[c]["out"] for c in range(NCORES)])
    return _unshard(outv)



# revision 13
# speedup vs baseline: 1.2369x; 1.2369x over previous
"""Trainium2 Bass kernel for nn_AssociationScore (GCN + MLP scoring head).

The computation is linear up to the final sigmoid, so the 64-dim GCN
aggregation collapses to a per-node scalar:
    w3  = W @ w2                       (256-vector, computed on device)
    u   = x @ w3                       (per-node scalar; memory-bound matvec)
    g   = u * dinv                     (dinv = rsqrt(indeg + 1))
    z0[d] = sum over edges (s->d) of g[s]
    score = sigmoid(dinv*z0 + dinv^2*u + (b@w2 + b2))

Sharding: nodes row-sharded over 8 NeuronCores (12500/core). Each core
computes its u/g shard, all-gathers g, then aggregates the edges whose dst
lies in its shard. The scatter-sum runs as: per-source-octant GPSIMD
ap_gather of g (8 independent index streams, one per Q7 core) -> DVE prefix
scan along the free dim -> second ap_gather at segment-boundary positions ->
differences. Host-side work is integer routing only (sort + CSR offsets +
int16 index tables); all floating-point math runs on device.
"""
import numpy as np

NCORES = 8
N = 100000
M = 12500            # nodes per core
TILES = 98           # 98 * 128 = 12544
MP = TILES * 128
D = 256
H = 64
NCH = 8              # gather/scan pipeline chunks
TB = 7               # matvec tiles per block
_GATE = [0] * 7      # set by _routing: scan chunk gating each extraction (98 = 14*7)


def _routing(src, dst):
    """Integer routing tables. Returns per-core wrapped int16 index arrays and sizes."""
    core = dst // M
    octv = src // M
    key = (core * 8 + octv) * np.int64(N) + dst
    order = np.argsort(key, kind='stable')
    s_s, d_s = src[order], dst[order]
    grp = (core * 8 + octv)[order]
    counts = np.bincount(grp, minlength=64)
    KE = int(counts.max()) + 1                      # +1 sentinel at stream head
    KE = ((KE + NCH * 64 - 1) // (NCH * 64)) * (NCH * 64)
    assert KE <= 32768
    offs = np.concatenate([[0], np.cumsum(counts)])
    idx_main = np.full((NCORES, 8, KE), M, np.int16)    # sentinel idx = M (zero pad row)
    ends = np.zeros((NCORES, 8, MP), np.int64)
    for c in range(NCORES):
        for o in range(8):
            gi = c * 8 + o
            L = counts[gi]
            sl = slice(offs[gi], offs[gi] + L)
            idx_main[c, o, 1:1 + L] = (s_s[sl] - o * M).astype(np.int16)
            dl = d_s[sl] - c * M
            cnt = np.bincount(dl, minlength=M)
            e = np.cumsum(cnt)                      # stream pos of last edge with dst<=n
            ends[c, o, :M] = e
            ends[c, o, M:] = e[-1]
    EKE = ((MP + TILES + 63) // 64) * 64
    ext = np.zeros((NCORES, 8, EKE), np.int64)
    ext[:, :, :MP] = ends
    for j in range(1, TILES):
        ext[:, :, MP + j] = ends[:, :, 128 * j - 1]
    deg = (np.bincount(dst, minlength=N) + 1).astype(np.float32)
    # extraction gating: node-chunk j (1792 nodes, 14 tiles) may extract only
    # after the scan chunk covering its largest end position completes
    CH = KE // NCH
    global _GATE
    _GATE = [max(0, int(ends[:, :, 1792 * (j + 1) - 1].max() - 1) // CH)
             for j in range(7)]

    def wrap16(a):                                  # [8, K] -> [128, K//16]
        o, K = a.shape
        return np.ascontiguousarray(
            a.reshape(o, K // 16, 16).transpose(0, 2, 1).reshape(o * 16, K // 16))

    idxm = np.stack([wrap16(idx_main[c]) for c in range(NCORES)])
    idxe = np.stack([wrap16(ext[c].astype(np.int16)) for c in range(NCORES)])
    return idxm, idxe, deg, KE, EKE


def _emit(nc, tc, t, KE, EKE, variant=9):
    """Emit the per-core program. `t` maps tensor names to DRAM APs."""
    import concourse.mybir as mybir
    import concourse.tile as tile  # noqa: F401

    dt = mybir.dt
    f32 = dt.float32
    Alu = mybir.AluOpType

    with tc.tile_pool(name="const", bufs=1) as cpool, \
         tc.tile_pool(name="big", bufs=1) as bpool, \
         tc.tile_pool(name="scr", bufs=1) as spool, \
         tc.tile_pool(name="gchp", bufs=2) as gpool, \
         tc.tile_pool(name="ps", bufs=1, space="PSUM") as ppool, \
         tc.tile_pool(name="psp", bufs=2, space="PSUM") as prodpool, \
         tc.tile_pool(name="dram", bufs=1, space="DRAM") as dpool:

        # ---- constants
        identt = cpool.tile([128, 128], f32)
        nc.sync.dma_start(identt[:], t["ident"])
        mselt = cpool.tile([128, 128], f32)
        nc.sync.dma_start(mselt[:], t["msel"])
        wTt = cpool.tile([H, D], f32)
        nc.sync.dma_start(wTt[:], t["wT"])
        w2rt = cpool.tile([H, 128], f32)
        nc.sync.dma_start(w2rt[:], t["w2r"])
        bcolt = cpool.tile([H, 1], f32)
        nc.sync.dma_start(bcolt[:], t["bcol"])
        b2rt = cpool.tile([128, 1], f32)
        nc.sync.dma_start(b2rt[:], t["b2r"])
        idxmt = cpool.tile([128, KE // 16], dt.int16)
        nc.sync.dma_start(idxmt[:], t["idxm"])
        idxet = cpool.tile([128, EKE // 16], dt.int16)
        nc.sync.dma_start(idxet[:], t["idxe"])
        degt = cpool.tile([TILES, 128], f32)
        nc.sync.dma_start(degt[:], t["deg2d"])

        if variant <= 0:
            nc.sync.dma_start(t["out"].rearrange("(a b) -> a b", a=TILES), degt[:])
            return

        # ---- w3 replicated [128, D]: out[p,k] = sum_j w2r[j,p]*wT[j,k]
        w3ps = ppool.tile([128, D], f32, tag="w3ps")
        nc.tensor.matmul(w3ps[:], w2rt[:], wTt[:], start=True, stop=True)
        w3rep = cpool.tile([128, D], f32)
        nc.vector.tensor_copy(w3rep[:], w3ps[:])
        if variant == 1:   # w3rep matmul only
            nc.sync.dma_start(t["out"].rearrange("(a b) -> a b", a=TILES), w3rep[0:TILES, 0:128])
            return

        # ---- c0 = b@w2 + b2, replicated [128, 1]
        c0ps = ppool.tile([128, 1], f32, tag="c0ps")
        nc.tensor.matmul(c0ps[:], w2rt[:], bcolt[:], start=True, stop=True)
        if variant == 11:   # + c0 matmul, no add
            nc.sync.dma_start(t["out"].rearrange("(a b) -> a b", a=TILES), w3rep[0:TILES, 0:128])
            return
        c0t = cpool.tile([128, 1], f32)
        nc.vector.tensor_add(c0t[:], c0ps[:], b2rt[:])
        if variant == 12:   # + c0 add
            nc.sync.dma_start(t["out"].rearrange("(a b) -> a b", a=TILES), w3rep[0:TILES, 0:128])
            return

        if variant == 2:   # consts + matmul w3rep only
            res0 = spool.tile([128, D], f32, tag="prodc")
            nc.vector.tensor_copy(res0[:], w3rep[:])
            nc.sync.dma_start(t["out"].rearrange("(a b) -> a b", a=TILES), res0[0:TILES, 0:128])
            return

        # ---- matvec u = xs @ w3 -> U [128, TILES]  (TB tiles per block)
        U = bpool.tile([128, 128], f32)
        nc.vector.memset(U[:], 0.0)
        NB = 1 if variant == 3 else TILES // TB
        xsv = t["xs"].rearrange("(b a p) k -> b p a k", p=128, a=TB)
        with tc.tile_pool(name="xload", bufs=2) as xpool:
            for j in range(NB):
                xt = xpool.tile([128, TB * D], f32, tag="xt")
                nc.sync.dma_start(
                    xt[:].rearrange("p (a k) -> p a k", k=D), xsv[j])
                xt3 = xt[:].rearrange("p (a k) -> p a k", k=D)
                nc.vector.tensor_mul(
                    xt3, xt3,
                    w3rep[:].rearrange("p (o k) -> p o k", o=1)
                    .broadcast_to([128, TB, D]))
                nc.vector.tensor_reduce(
                    U[:, j * TB:(j + 1) * TB],
                    xt[:].rearrange("p (a k) -> p a k", k=D),
                    axis=mybir.AxisListType.X, op=Alu.add)

        if variant in (3, 4):   # matvec only (1 tile or all)
            nc.sync.dma_start(t["out"].rearrange("(a b) -> a b", a=TILES), U[0:TILES, :])
            return

        # ---- dinv = sqrt(1/deg) [TILES, 128]
        rec = spool.tile([TILES, 128], f32, tag="rec")
        nc.vector.reciprocal(rec[:], degt[:])
        dinvt = bpool.tile([TILES, 128], f32)
        nc.scalar.sqrt(dinvt[:], rec[:])

        # ---- transpose U -> [TILES, 128]; g = dinv * uT
        ups = ppool.tile([128, 128], f32, tag="ups")
        nc.tensor.transpose(ups[:], U[:], identt[:])
        gblk = bpool.tile([TILES, 128], f32)
        nc.vector.tensor_mul(gblk[:], dinvt[:], ups[0:TILES, :])

        if variant <= 5:
            res1 = spool.tile([TILES, 128], f32, tag="res")
            nc.scalar.activation(res1[:], gblk[:],
                                 mybir.ActivationFunctionType.Sigmoid,
                                 bias=c0t[0:TILES, :])
            nc.sync.dma_start(t["out"].rearrange("(a b) -> a b", a=TILES), res1[:])
            return

        # ---- allgather g
        gshard = dpool.tile([1, MP], f32)
        nc.sync.dma_start(
            gshard.opt()[0].rearrange("(a b) -> a b", a=TILES), gblk[:])
        gfull = dpool.tile([NCORES, MP], f32)
        nc.gpsimd.collective_compute(
            "AllGather", Alu.bypass,
            replica_groups=[list(range(NCORES))],
            ins=[gshard.opt()], outs=[gfull.opt()])

        # ---- octant tables [128, MP]: partition group o holds shard o (x16)
        gtab = bpool.tile([128, MP], f32, tag="gtb")
        for o in range(8):
            try:
                src = gfull.opt()[o:o + 1, :].broadcast_to([16, MP])
                nc.sync.dma_start(gtab[16 * o:16 * (o + 1), :], src)
            except Exception:
                for r in range(16):
                    nc.sync.dma_start(
                        gtab[16 * o + r:16 * o + r + 1, :],
                        gfull.opt()[o:o + 1, :])

        if variant <= 6:
            res2 = spool.tile([TILES, 128], f32, tag="res")
            nc.vector.tensor_copy(res2[:], gtab[0:TILES, 0:128])
            nc.sync.dma_start(t["out"].rearrange("(a b) -> a b", a=TILES), res2[:])
            return

        # ---- gather + prefix scan over edge streams; the boundary
        # extraction runs in 7 node-chunks (1792 nodes = 14 tiles each),
        # gated on scan progress and interleaved with the gathers. Octant
        # partials are summed on the PE via a row-selector matmul, so no
        # DRAM bounce is needed.
        scn = bpool.tile([128, KE], f32)
        ones1 = cpool.tile([128, 1], f32)
        nc.vector.memset(ones1[:], 1.0)
        CH = KE // NCH
        EXN = 1792
        scn3 = scn[:].rearrange("p (n d) -> p n d", d=1)
        gtab3 = gtab[:].rearrange("p (n d) -> p n d", d=1)
        Zt = bpool.tile([TILES, 128], f32)
        zred = []
        for _zj in range(7):
            zr = dpool.tile([1, 1792], f32, tag=f"zred{_zj}")
            zred.append(zr)

        def emit_ext(j):
            BOj = gpool.tile([128, CH], f32, tag="gch")
            nc.gpsimd.ap_gather(
                out_ap=BOj[:, 0:EXN].rearrange("p (n d) -> p n d", d=1),
                in_ap=scn3,
                idxs_ap=idxet[:, (EXN // 16) * j:(EXN // 16) * (j + 1)],
                channels=128, num_elems=KE, d=1,
                num_idxs=EXN)
            # sum the 8 octant rows (partitions 16o) of each 128-col block:
            # out[p, f] = sum_c msel[c, p] * BOj[c, f] = sum_o BOj[16o, f]
            ZSj = gpool.tile([128, CH], f32, tag="gch")
            for m2 in range(EXN // 128):
                zm = prodpool.tile([128, 128], f32, tag="zm")
                nc.tensor.matmul(zm[:], mselt[:],
                                 BOj[:, 128 * m2:128 * (m2 + 1)],
                                 start=True, stop=True)
                nc.vector.tensor_copy(ZSj[:, 128 * m2:128 * (m2 + 1)], zm[:])
            # redistribute [1, 1792] (rows identical) -> Zt rows [14j, 14j+14)
            # via a whole-tile DRAM bounce (slice-level DRAM deps misorder on
            # this runtime; whole-tile write->read is the proven pattern)
            nc.sync.dma_start(zred[j].opt()[0:1, :], ZSj[0:1, 0:EXN])
            nc.sync.dma_start(
                Zt[14 * j:14 * (j + 1), :],
                zred[j].opt()[0].rearrange("(a b) -> a b", b=128))

        nxj = 0
        for tt in range(NCH):
            gch = gpool.tile([128, CH], f32, tag="gch")
            nc.gpsimd.ap_gather(
                out_ap=gch[:].rearrange("p (n d) -> p n d", d=1),
                in_ap=gtab3,
                idxs_ap=idxmt[:, tt * (CH // 16):(tt + 1) * (CH // 16)],
                channels=128, num_elems=MP, d=1,
                num_idxs=CH)
            if variant <= 7:
                res3 = spool.tile([TILES, 128], f32, tag="res")
                nc.vector.tensor_copy(res3[:], gch[0:TILES, 0:128])
                nc.sync.dma_start(t["out"].rearrange("(a b) -> a b", a=TILES), res3[:])
                return
            init = 0.0 if tt == 0 else scn[:, tt * CH - 1:tt * CH]
            nc.vector.tensor_tensor_scan(
                out=scn[:, tt * CH:(tt + 1) * CH],
                data0=ones1[:].broadcast_to([128, CH]),
                data1=gch[:],
                initial=init, op0=Alu.mult, op1=Alu.add)
            if variant > 8:
                while nxj < 7 and _GATE[nxj] <= tt - 1:
                    emit_ext(nxj)
                    nxj += 1

        if variant <= 8:
            res4 = spool.tile([TILES, 128], f32, tag="res")
            nc.vector.tensor_copy(res4[:], scn[0:TILES, 0:128])
            nc.sync.dma_start(t["out"].rearrange("(a b) -> a b", a=TILES), res4[:])
            return

        while nxj < 7:
            emit_ext(nxj)
            nxj += 1

        # ---- diff, combine, sigmoid. Tile-row boundary values come from a
        # one-column shift DMA of Zt itself (Zx[t] = Zt[t-1, 127], Zx[0] = 0).
        Zx = spool.tile([TILES, 1], f32, tag="zx")
        nc.vector.memset(Zx[:], 0.0)
        nc.sync.dma_start(Zx[1:TILES, 0:1], Zt[0:TILES - 1, 127:128])
        Dt = spool.tile([TILES, 128], f32, tag="dt")
        nc.vector.tensor_sub(Dt[:, 1:128], Zt[:, 1:128], Zt[:, 0:127])
        nc.vector.tensor_sub(Dt[:, 0:1], Zt[:, 0:1], Zx[:])
        t1 = spool.tile([TILES, 128], f32, tag="t1")
        nc.vector.tensor_add(t1[:], Dt[:], gblk[:])
        t2 = spool.tile([TILES, 128], f32, tag="t2")
        nc.vector.tensor_mul(t2[:], dinvt[:], t1[:])
        res = spool.tile([TILES, 128], f32, tag="res")
        nc.scalar.activation(res[:], t2[:],
                             mybir.ActivationFunctionType.Sigmoid,
                             bias=c0t[0:TILES, :])
        nc.sync.dma_start(t["out"].rearrange("(a b) -> a b", a=TILES), res[:])


def _build_nc(KE, EKE, variant=9):
    import concourse.bacc as bacc
    import concourse.mybir as mybir
    import concourse.tile as tile

    dt = mybir.dt
    f32 = dt.float32
    nc = bacc.Bacc("TRN2", target_bir_lowering=False, debug=False,
                   num_devices=NCORES)
    t = {
        "xs": nc.dram_tensor("xs", [MP, D], f32, kind="ExternalInput").ap(),
        "wT": nc.dram_tensor("wT", [H, D], f32, kind="ExternalInput").ap(),
        "w2r": nc.dram_tensor("w2r", [H, 128], f32, kind="ExternalInput").ap(),
        "bcol": nc.dram_tensor("bcol", [H, 1], f32, kind="ExternalInput").ap(),
        "b2r": nc.dram_tensor("b2r", [128, 1], f32, kind="ExternalInput").ap(),
        "deg2d": nc.dram_tensor("deg2d", [TILES, 128], f32, kind="ExternalInput").ap(),
        "ident": nc.dram_tensor("ident", [128, 128], f32, kind="ExternalInput").ap(),
        "msel": nc.dram_tensor("msel", [128, 128], f32, kind="ExternalInput").ap(),
        "idxm": nc.dram_tensor("idxm", [128, KE // 16], dt.int16, kind="ExternalInput").ap(),
        "idxe": nc.dram_tensor("idxe", [128, EKE // 16], dt.int16, kind="ExternalInput").ap(),
        "out": nc.dram_tensor("out", [MP], f32, kind="ExternalOutput").ap(),
    }
    with tile.TileContext(nc) as tc:
        _emit(nc, tc, t, KE, EKE, variant)
    nc.compile()
    return nc


def _make_in_maps(x, edge_index, W, b, w2, b2):
    src = np.asarray(edge_index[0], dtype=np.int64)
    dst = np.asarray(edge_index[1], dtype=np.int64)
    idxm, idxe, deg, KE, EKE = _routing(src, dst)

    xf = np.asarray(x, dtype=np.float32)
    Wf = np.asarray(W, dtype=np.float32)
    w2f = np.asarray(w2, dtype=np.float32).reshape(H)
    bf = np.asarray(b, dtype=np.float32)
    b2f = np.asarray(b2, dtype=np.float32).reshape(1)

    wT = np.ascontiguousarray(Wf.T)                       # [H, D]
    w2rep = np.ascontiguousarray(np.broadcast_to(w2f.reshape(H, 1), (H, 128)))
    bcol = bf.reshape(H, 1)
    b2rep = np.full((128, 1), float(b2f[0]), np.float32)
    identm = np.eye(128, dtype=np.float32)
    mselm = np.zeros((128, 128), np.float32)
    mselm[0::16, :] = 1.0          # matmul(out, msel, B): out[p,f]=sum_o B[16o,f]

    in_maps = []
    for c in range(NCORES):
        xsp = np.zeros((MP, D), np.float32)
        xsp[:M] = xf[c * M:(c + 1) * M]
        degp = np.ones(MP, np.float32)
        degp[:M] = deg[c * M:(c + 1) * M]
        in_maps.append({
            "xs": xsp,
            "wT": wT,
            "w2r": w2rep,
            "bcol": bcol,
            "b2r": b2rep,
            "deg2d": degp.reshape(TILES, 128),
            "ident": identm,
            "msel": mselm,
            "idxm": idxm[c],
            "idxe": idxe[c],
        })
    return in_maps, KE, EKE


def _unshard(outv):
    return np.concatenate([outv[c][:M] for c in range(NCORES)]).astype(np.float32)


def kernel(x, edge_index, W, b, w2, b2):
    in_maps, KE, EKE = _make_in_maps(x, edge_index, W, b, w2, b2)
    nc = _build_nc(KE, EKE)
    from concourse.bass_utils import run_bass_kernel_spmd
    res = None
    for attempt in range(3):
        try:
            res = run_bass_kernel_spmd(nc, in_maps, list(range(NCORES)))
            break
        except Exception:
            if attempt == 2:
                raise
            # device may be in a bad state from a prior run; exercise the
            # plain XLA path on every core to recover, then retry
            import jax
            import jax.numpy as jnp
            a = np.eye(128, dtype=np.float32)
            for d in jax.devices()[:NCORES]:
                jnp.dot(jax.device_put(a, d), jax.device_put(a, d)).block_until_ready()
    outv = np.stack([res.results[c]["out"] for c in range(NCORES)])
    return _unshard(outv)



# revision 15
# speedup vs baseline: 1.5881x; 1.2840x over previous
"""Trainium2 Bass kernel for nn_AssociationScore (GCN + MLP scoring head).

The computation is linear up to the final sigmoid, so the 64-dim GCN
aggregation collapses to a per-node scalar:
    w3  = W @ w2                       (256-vector, computed on device)
    u   = x @ w3                       (per-node scalar; memory-bound matvec)
    g   = u * dinv                     (dinv = rsqrt(indeg + 1))
    z0[d] = sum over edges (s->d) of g[s]
    score = sigmoid(dinv*z0 + dinv^2*u + (b@w2 + b2))

Sharding: nodes row-sharded over 8 NeuronCores (12500/core). Each core
computes its u/g shard, all-gathers g, then aggregates the edges whose dst
lies in its shard. The scatter-sum runs as: per-source-octant GPSIMD
ap_gather of g (8 independent index streams, one per Q7 core) -> DVE prefix
scan along the free dim -> second ap_gather at segment-boundary positions ->
differences. Host-side work is integer routing only (sort + CSR offsets +
int16 index tables); all floating-point math runs on device.
"""
import numpy as np

NCORES = 8
N = 100000
M = 12500            # nodes per core
TILES = 98           # 98 * 128 = 12544
MP = TILES * 128
D = 256
H = 64
NCH = 8              # gather/scan pipeline chunks
TB = 7               # matvec tiles per block (98 = 14*7)


def _routing(src, dst):
    """Integer routing tables. Returns per-core wrapped int16 index arrays and sizes."""
    core = dst // M
    octv = src // M
    key = (core * 8 + octv) * np.int64(N) + dst
    order = np.argsort(key, kind='stable')
    s_s, d_s = src[order], dst[order]
    grp = (core * 8 + octv)[order]
    counts = np.bincount(grp, minlength=64)
    KE = int(counts.max()) + 1                      # +1 sentinel at stream head
    KE = ((KE + NCH * 64 - 1) // (NCH * 64)) * (NCH * 64)
    assert KE <= 32768
    offs = np.concatenate([[0], np.cumsum(counts)])
    idx_main = np.full((NCORES, 8, KE), M, np.int16)    # sentinel idx = M (zero pad row)
    ends = np.zeros((NCORES, 8, MP), np.int64)
    for c in range(NCORES):
        for o in range(8):
            gi = c * 8 + o
            L = counts[gi]
            sl = slice(offs[gi], offs[gi] + L)
            idx_main[c, o, 1:1 + L] = (s_s[sl] - o * M).astype(np.int16)
            dl = d_s[sl] - c * M
            cnt = np.bincount(dl, minlength=M)
            e = np.cumsum(cnt)                      # stream pos of last edge with dst<=n
            ends[c, o, :M] = e
            ends[c, o, M:] = e[-1]
    EKE = ((MP + TILES + 63) // 64) * 64
    ext = np.zeros((NCORES, 8, EKE), np.int64)
    ext[:, :, :MP] = ends
    for j in range(1, TILES):
        ext[:, :, MP + j] = ends[:, :, 128 * j - 1]
    deg = (np.bincount(dst, minlength=N) + 1).astype(np.float32)

    def wrap16(a):                                  # [8, K] -> [128, K//16]
        o, K = a.shape
        return np.ascontiguousarray(
            a.reshape(o, K // 16, 16).transpose(0, 2, 1).reshape(o * 16, K // 16))

    idxm = np.stack([wrap16(idx_main[c]) for c in range(NCORES)])
    idxe = np.stack([wrap16(ext[c].astype(np.int16)) for c in range(NCORES)])
    return idxm, idxe, deg, KE, EKE


def _emit(nc, tc, t, KE, EKE, variant=9):
    """Emit the per-core program. `t` maps tensor names to DRAM APs."""
    import concourse.mybir as mybir
    import concourse.tile as tile  # noqa: F401

    dt = mybir.dt
    f32 = dt.float32
    Alu = mybir.AluOpType

    with tc.tile_pool(name="const", bufs=1) as cpool, \
         tc.tile_pool(name="big", bufs=1) as bpool, \
         tc.tile_pool(name="xload", bufs=2) as xpool, \
         tc.tile_pool(name="scr", bufs=1) as spool, \
         tc.tile_pool(name="gchp", bufs=2) as gpool, \
         tc.tile_pool(name="ps", bufs=1, space="PSUM") as ppool, \
         tc.tile_pool(name="psp", bufs=2, space="PSUM") as prodpool, \
         tc.tile_pool(name="dram", bufs=1, space="DRAM") as dpool:

        # ---- constants
        identt = cpool.tile([128, 128], f32)
        nc.sync.dma_start(identt[:], t["ident"])
        wTt = cpool.tile([H, D], f32)
        nc.sync.dma_start(wTt[:], t["wT"])
        w2rt = cpool.tile([H, 128], f32)
        nc.sync.dma_start(w2rt[:], t["w2r"])
        bcolt = cpool.tile([H, 1], f32)
        nc.sync.dma_start(bcolt[:], t["bcol"])
        b2rt = cpool.tile([128, 1], f32)
        nc.sync.dma_start(b2rt[:], t["b2r"])
        idxmt = cpool.tile([128, KE // 16], dt.int16)
        nc.sync.dma_start(idxmt[:], t["idxm"])
        idxet = cpool.tile([128, EKE // 16], dt.int16)
        nc.sync.dma_start(idxet[:], t["idxe"])
        degt = cpool.tile([TILES, 128], f32)
        nc.sync.dma_start(degt[:], t["deg2d"])

        if variant <= 0:
            nc.sync.dma_start(t["out"].rearrange("(a b) -> a b", a=TILES), degt[:])
            return

        # ---- w3 replicated [128, D]: out[p,k] = sum_j w2r[j,p]*wT[j,k]
        w3ps = ppool.tile([128, D], f32, tag="w3ps")
        nc.tensor.matmul(w3ps[:], w2rt[:], wTt[:], start=True, stop=True)
        w3rep = cpool.tile([128, D], f32)
        nc.vector.tensor_copy(w3rep[:], w3ps[:])
        if variant == 1:   # w3rep matmul only
            nc.sync.dma_start(t["out"].rearrange("(a b) -> a b", a=TILES), w3rep[0:TILES, 0:128])
            return

        # ---- c0 = b@w2 + b2, replicated [128, 1]
        c0ps = ppool.tile([128, 1], f32, tag="c0ps")
        nc.tensor.matmul(c0ps[:], w2rt[:], bcolt[:], start=True, stop=True)
        if variant == 11:   # + c0 matmul, no add
            nc.sync.dma_start(t["out"].rearrange("(a b) -> a b", a=TILES), w3rep[0:TILES, 0:128])
            return
        c0t = cpool.tile([128, 1], f32)
        nc.vector.tensor_add(c0t[:], c0ps[:], b2rt[:])
        if variant == 12:   # + c0 add
            nc.sync.dma_start(t["out"].rearrange("(a b) -> a b", a=TILES), w3rep[0:TILES, 0:128])
            return

        if variant == 2:   # consts + matmul w3rep only
            res0 = spool.tile([128, D], f32, tag="prodc")
            nc.vector.tensor_copy(res0[:], w3rep[:])
            nc.sync.dma_start(t["out"].rearrange("(a b) -> a b", a=TILES), res0[0:TILES, 0:128])
            return

        # ---- matvec u = x @ w3 on the PE: per 128-node tile, two
        # accumulating matmuls contract the 256-dim in 128-chunks.
        # xs is host-transposed bf16 [D, MP]; out[p, f] = sum_c A[c,p]B[c,f].
        U = bpool.tile([128, 128], f32)
        nc.vector.memset(U[:], 0.0)
        w3c0ps = ppool.tile([128, 1], f32, tag="w3c0")
        nc.tensor.matmul(w3c0ps[:], wTt[:, 0:128], w2rt[:, 0:1],
                         start=True, stop=True)
        w3c1ps = ppool.tile([128, 1], f32, tag="w3c1")
        nc.tensor.matmul(w3c1ps[:], wTt[:, 128:256], w2rt[:, 0:1],
                         start=True, stop=True)
        w3c0 = cpool.tile([128, 1], dt.bfloat16)
        nc.vector.tensor_copy(w3c0[:], w3c0ps[:])
        w3c1 = cpool.tile([128, 1], dt.bfloat16)
        nc.vector.tensor_copy(w3c1[:], w3c1ps[:])
        ups2 = ppool.tile([128, TILES], f32, tag="ups2")
        NB = 1 if variant == 3 else TILES // TB
        xsv = t["xs"].rearrange("(kc k) (b n) -> kc b k n", k=128, n=TB * 128)
        with tc.tile_pool(name="xload", bufs=3) as xpool:
            for b in range(NB):
                xta = xpool.tile([128, TB * 128], dt.bfloat16, tag="xta")
                nc.sync.dma_start(xta[:], xsv[0][b])
                xtb = xpool.tile([128, TB * 128], dt.bfloat16, tag="xtb")
                nc.sync.dma_start(xtb[:], xsv[1][b])
                for a in range(TB):
                    col = b * TB + a
                    nc.tensor.matmul(
                        ups2[:, col:col + 1],
                        xta[:, 128 * a:128 * (a + 1)], w3c0[:],
                        start=True, stop=False)
                    nc.tensor.matmul(
                        ups2[:, col:col + 1],
                        xtb[:, 128 * a:128 * (a + 1)], w3c1[:],
                        start=False, stop=True)
        nc.vector.tensor_copy(U[:, 0:TILES], ups2[:])

        if variant in (3, 4):   # matvec only (1 tile or all)
            nc.sync.dma_start(t["out"].rearrange("(a b) -> a b", a=TILES), U[0:TILES, :])
            return

        # ---- dinv = sqrt(1/deg) [TILES, 128]
        rec = spool.tile([TILES, 128], f32, tag="rec")
        nc.vector.reciprocal(rec[:], degt[:])
        dinvt = bpool.tile([TILES, 128], f32)
        nc.scalar.sqrt(dinvt[:], rec[:])

        # ---- transpose U -> [TILES, 128]; g = dinv * uT
        ups = ppool.tile([128, 128], f32, tag="ups")
        nc.tensor.transpose(ups[:], U[:], identt[:])
        gblk = bpool.tile([TILES, 128], f32)
        nc.vector.tensor_mul(gblk[:], dinvt[:], ups[0:TILES, :])

        if variant <= 5:
            res1 = spool.tile([TILES, 128], f32, tag="res")
            nc.scalar.activation(res1[:], gblk[:],
                                 mybir.ActivationFunctionType.Sigmoid,
                                 bias=c0t[0:TILES, :])
            nc.sync.dma_start(t["out"].rearrange("(a b) -> a b", a=TILES), res1[:])
            return

        # ---- allgather g
        gshard = dpool.tile([1, MP], f32)
        nc.sync.dma_start(
            gshard.opt()[0].rearrange("(a b) -> a b", a=TILES), gblk[:])
        gfull = dpool.tile([NCORES, MP], f32)
        nc.gpsimd.collective_compute(
            "AllGather", Alu.bypass,
            replica_groups=[list(range(NCORES))],
            ins=[gshard.opt()], outs=[gfull.opt()])

        # ---- octant tables [128, MP]: partition group o holds shard o (x16)
        gtab = bpool.tile([128, MP], f32, tag="gtb")
        for o in range(8):
            try:
                src = gfull.opt()[o:o + 1, :].broadcast_to([16, MP])
                nc.sync.dma_start(gtab[16 * o:16 * (o + 1), :], src)
            except Exception:
                for r in range(16):
                    nc.sync.dma_start(
                        gtab[16 * o + r:16 * o + r + 1, :],
                        gfull.opt()[o:o + 1, :])

        if variant <= 6:
            res2 = spool.tile([TILES, 128], f32, tag="res")
            nc.vector.tensor_copy(res2[:], gtab[0:TILES, 0:128])
            nc.sync.dma_start(t["out"].rearrange("(a b) -> a b", a=TILES), res2[:])
            return

        # ---- gather + prefix scan over edge streams
        scn = bpool.tile([128, KE], f32)
        ones1 = cpool.tile([128, 1], f32)
        nc.vector.memset(ones1[:], 1.0)
        CH = KE // NCH
        gtab3 = gtab[:].rearrange("p (n d) -> p n d", d=1)
        for tt in range(NCH):
            gch = gpool.tile([128, CH], f32, tag="gch")
            nc.gpsimd.ap_gather(
                out_ap=gch[:].rearrange("p (n d) -> p n d", d=1),
                in_ap=gtab3,
                idxs_ap=idxmt[:, tt * (CH // 16):(tt + 1) * (CH // 16)],
                channels=128, num_elems=MP, d=1,
                num_idxs=CH)
            if variant <= 7:
                res3 = spool.tile([TILES, 128], f32, tag="res")
                nc.vector.tensor_copy(res3[:], gch[0:TILES, 0:128])
                nc.sync.dma_start(t["out"].rearrange("(a b) -> a b", a=TILES), res3[:])
                return
            init = 0.0 if tt == 0 else scn[:, tt * CH - 1:tt * CH]
            nc.vector.tensor_tensor_scan(
                out=scn[:, tt * CH:(tt + 1) * CH],
                data0=ones1[:].broadcast_to([128, CH]),
                data1=gch[:],
                initial=init, op0=Alu.mult, op1=Alu.add)

        if variant <= 8:
            res4 = spool.tile([TILES, 128], f32, tag="res")
            nc.vector.tensor_copy(res4[:], scn[0:TILES, 0:128])
            nc.sync.dma_start(t["out"].rearrange("(a b) -> a b", a=TILES), res4[:])
            return

        # ---- boundary extraction
        bexf = bpool.tile([128, max(MP, EKE)], f32, tag="gtb")
        bex = bexf[:, 0:EKE]
        nc.gpsimd.ap_gather(
            out_ap=bex[:].rearrange("p (n d) -> p n d", d=1),
            in_ap=scn[:].rearrange("p (n d) -> p n d", d=1),
            idxs_ap=idxet[:],
            channels=128, num_elems=KE, d=1,
            num_idxs=EKE)

        # ---- redistribute per octant to block layout via DRAM bounce
        bexd = dpool.tile([8, EKE], f32)
        nc.sync.dma_start(bexd.opt()[:, :], bexf[0:128:16, 0:EKE])
        Ball = bpool.tile([TILES, 8 * 128], f32)
        Bx = bpool.tile([TILES, 8], f32)
        nc.sync.dma_start(
            Ball[:].rearrange("j (o c) -> j o c", o=8),
            bexd.opt()[:, 0:MP].rearrange("o (j c) -> j o c", j=TILES))
        nc.sync.dma_start(
            Bx[:],
            bexd.opt()[:, MP:MP + TILES].rearrange("o j -> j o"))

        # ---- merge octants, diff, combine, sigmoid
        Zt = spool.tile([TILES, 128], f32, tag="zt")
        nc.vector.tensor_add(Zt[:], Ball[:, 0:128], Ball[:, 128:256])
        for o in range(2, 8):
            nc.vector.tensor_add(Zt[:], Zt[:], Ball[:, o * 128:(o + 1) * 128])
        Zx = spool.tile([TILES, 1], f32, tag="zx")
        nc.vector.tensor_reduce(Zx[:], Bx[:], axis=mybir.AxisListType.X,
                                op=Alu.add)
        Dt = spool.tile([TILES, 128], f32, tag="dt")
        nc.vector.tensor_sub(Dt[:, 1:128], Zt[:, 1:128], Zt[:, 0:127])
        nc.vector.tensor_sub(Dt[:, 0:1], Zt[:, 0:1], Zx[:])
        t1 = spool.tile([TILES, 128], f32, tag="t1")
        nc.vector.tensor_add(t1[:], Dt[:], gblk[:])
        t2 = spool.tile([TILES, 128], f32, tag="t2")
        nc.vector.tensor_mul(t2[:], dinvt[:], t1[:])
        res = spool.tile([TILES, 128], f32, tag="res")
        nc.scalar.activation(res[:], t2[:],
                             mybir.ActivationFunctionType.Sigmoid,
                             bias=c0t[0:TILES, :])
        nc.sync.dma_start(t["out"].rearrange("(a b) -> a b", a=TILES), res[:])


def _build_nc(KE, EKE, variant=9):
    import concourse.bacc as bacc
    import concourse.mybir as mybir
    import concourse.tile as tile

    dt = mybir.dt
    f32 = dt.float32
    nc = bacc.Bacc("TRN2", target_bir_lowering=False, debug=False,
                   num_devices=NCORES)
    t = {
        "xs": nc.dram_tensor("xs", [D, MP], dt.bfloat16, kind="ExternalInput").ap(),
        "wT": nc.dram_tensor("wT", [H, D], f32, kind="ExternalInput").ap(),
        "w2r": nc.dram_tensor("w2r", [H, 128], f32, kind="ExternalInput").ap(),
        "bcol": nc.dram_tensor("bcol", [H, 1], f32, kind="ExternalInput").ap(),
        "b2r": nc.dram_tensor("b2r", [128, 1], f32, kind="ExternalInput").ap(),
        "deg2d": nc.dram_tensor("deg2d", [TILES, 128], f32, kind="ExternalInput").ap(),
        "ident": nc.dram_tensor("ident", [128, 128], f32, kind="ExternalInput").ap(),
        "idxm": nc.dram_tensor("idxm", [128, KE // 16], dt.int16, kind="ExternalInput").ap(),
        "idxe": nc.dram_tensor("idxe", [128, EKE // 16], dt.int16, kind="ExternalInput").ap(),
        "out": nc.dram_tensor("out", [MP], f32, kind="ExternalOutput").ap(),
    }
    with tile.TileContext(nc) as tc:
        _emit(nc, tc, t, KE, EKE, variant)
    nc.compile()
    return nc


def _make_in_maps(x, edge_index, W, b, w2, b2):
    src = np.asarray(edge_index[0], dtype=np.int64)
    dst = np.asarray(edge_index[1], dtype=np.int64)
    idxm, idxe, deg, KE, EKE = _routing(src, dst)

    xf = np.asarray(x, dtype=np.float32)
    Wf = np.asarray(W, dtype=np.float32)
    w2f = np.asarray(w2, dtype=np.float32).reshape(H)
    bf = np.asarray(b, dtype=np.float32)
    b2f = np.asarray(b2, dtype=np.float32).reshape(1)

    wT = np.ascontiguousarray(Wf.T)                       # [H, D]
    w2rep = np.ascontiguousarray(np.broadcast_to(w2f.reshape(H, 1), (H, 128)))
    bcol = bf.reshape(H, 1)
    b2rep = np.full((128, 1), float(b2f[0]), np.float32)
    identm = np.eye(128, dtype=np.float32)

    in_maps = []
    for c in range(NCORES):
        import ml_dtypes
        xsp = np.zeros((MP, D), np.float32)
        xsp[:M] = xf[c * M:(c + 1) * M]
        xsp = np.ascontiguousarray(xsp.T).astype(ml_dtypes.bfloat16)
        degp = np.ones(MP, np.float32)
        degp[:M] = deg[c * M:(c + 1) * M]
        in_maps.append({
            "xs": xsp,
            "wT": wT,
            "w2r": w2rep,
            "bcol": bcol,
            "b2r": b2rep,
            "deg2d": degp.reshape(TILES, 128),
            "ident": identm,
            "idxm": idxm[c],
            "idxe": idxe[c],
        })
    return in_maps, KE, EKE


def _unshard(outv):
    return np.concatenate([outv[c][:M] for c in range(NCORES)]).astype(np.float32)


def kernel(x, edge_index, W, b, w2, b2):
    in_maps, KE, EKE = _make_in_maps(x, edge_index, W, b, w2, b2)
    nc = _build_nc(KE, EKE)
    from concourse.bass_utils import run_bass_kernel_spmd
    res = None
    for attempt in range(3):
        try:
            res = run_bass_kernel_spmd(nc, in_maps, list(range(NCORES)))
            break
        except Exception:
            if attempt == 2:
                raise
            # device may be in a bad state from a prior run; exercise the
            # plain XLA path on every core to recover, then retry
            import jax
            import jax.numpy as jnp
            a = np.eye(128, dtype=np.float32)
            for d in jax.devices()[:NCORES]:
                jnp.dot(jax.device_put(a, d), jax.device_put(a, d)).block_until_ready()
    outv = np.stack([res.results[c]["out"] for c in range(NCORES)])
    return _unshard(outv)

